# revision 17
# baseline (speedup 1.0000x reference)
"""MoE router kernel for 8 trn2 NeuronCores.

Computes, for x [16384, 4096] f32, W [4096, 64] f32, b [64] f32:
    logits = x @ W + b
    probs  = softmax(logits)
    top2p, top2i = top_k(probs, 2);  top2p /= sum(top2p)
returns (top2p [16384,2] f32, top2i [16384,2] int32).

Sharding: tokens split across 8 cores (2048 each), W/b replicated.
Per-core pipeline:
  - DMA x rows naturally ([128 tok, 4096 d] tiles)
  - PE transpose 128x128 blocks -> xT [d, tok] (via identity matmul)
  - PSUM->SBUF copies (DVE/ACT alternating)
  - PE matmul logitsT[e, tok] = W.T @ xT accumulated over 32 d-tiles
  - bias folded into the PSUM->SBUF logits copy (per-partition scalar add)
  - small PE transpose back to [tok, e], DVE max8/max_index top-2,
    2-term softmax renorm (exp/sum/reciprocal), DMA out.

Constraint: walrus allows only ONE sync-wait on self-loading (fp32 /
transpose-mode) Matmult instructions.  The structure below keeps every
PE instruction at <=1 wait: prologue dummy PE ops absorb the
identity/W-load deps, and dt==0 transposes get a dedicated PSUM pool so
their bank-WAW dep is ancient (elided by Tile's wait analysis).
"""

import numpy as np

NUM_TOKENS = 16384
D_MODEL = 4096
NUM_EXPERTS = 64
TOP_K = 2
N_CORES = 8
TOK_PER_CORE = NUM_TOKENS // N_CORES  # 2048
TOK_BLOCK = 512
N_BLOCKS = TOK_PER_CORE // TOK_BLOCK  # 4
D_TILE = 128
N_DT = D_MODEL // D_TILE  # 32

# dtype knobs: "f32" (exact, 4 cyc/row matmul) or "f32r" (relaxed, ~1 cyc/row)
MM_DTYPE = "f32"
TR_DTYPE = "f32"
REPEAT = 1  # run the whole pipeline this many times in one NEFF (for timing slope)

_BUILT = {}


def _build_nc():
    import concourse.bass as bass
    import concourse.bacc as bacc
    import concourse.tile as tile
    from concourse import masks, mybir
    from contextlib import ExitStack

    f32 = mybir.dt.float32
    f32r = mybir.dt.float32r
    u32 = mybir.dt.uint32
    mm_dt = f32 if MM_DTYPE == "f32" else f32r
    tr_dt = f32 if TR_DTYPE == "f32" else f32r

    def mmv(ap):
        return ap.bitcast(mm_dt) if mm_dt is not f32 else ap

    def trv(ap):
        return ap.bitcast(tr_dt) if tr_dt is not f32 else ap

    nc = bacc.Bacc("TRN2", target_bir_lowering=False, debug=False, num_devices=N_CORES)

    x_ap = nc.dram_tensor("x", [TOK_PER_CORE, D_MODEL], f32, kind="ExternalInput").ap()
    # wp = W [4096, 64] host-permuted to [128, 32, 64] (partition-major d-tiles)
    # so the load is one contiguous-per-partition DMA instead of 4096 small
    # descriptors.
    w_ap = nc.dram_tensor(
        "wp", [128, N_DT * NUM_EXPERTS], f32, kind="ExternalInput"
    ).ap()
    b_ap = nc.dram_tensor("b", [1, NUM_EXPERTS], f32, kind="ExternalInput").ap()
    op_ap = nc.dram_tensor(
        "probs", [REPEAT * TOK_PER_CORE, TOP_K], f32, kind="ExternalOutput"
    ).ap()
    oi_ap = nc.dram_tensor(
        "idx", [REPEAT * TOK_PER_CORE, TOP_K], u32, kind="ExternalOutput"
    ).ap()

    n_tt = TOK_BLOCK // 128  # token 128-tiles per block

    with tile.TileContext(nc) as tc, ExitStack() as ctx:
        const_pool = ctx.enter_context(tc.tile_pool(name="const", bufs=1))
        xnat_pool = ctx.enter_context(tc.tile_pool(name="xnat", bufs=2 * n_tt))
        xT_ps0_pool = ctx.enter_context(tc.tile_pool(name="xT_ps0", bufs=1, space="PSUM"))
        xT_ps_pool = ctx.enter_context(tc.tile_pool(name="xT_ps", bufs=3, space="PSUM"))
        xT_sb_pool = ctx.enter_context(tc.tile_pool(name="xT_sb", bufs=3))
        lg_ps_pool = ctx.enter_context(tc.tile_pool(name="lg_ps", bufs=2, space="PSUM"))
        lg_sb_pool = ctx.enter_context(tc.tile_pool(name="lg_sb", bufs=2))
        lt_ps_pool = ctx.enter_context(tc.tile_pool(name="lt_ps", bufs=2, space="PSUM"))
        epi_pool = ctx.enter_context(tc.tile_pool(name="epi", bufs=2))
        out_pool = ctx.enter_context(tc.tile_pool(name="outs", bufs=2))

        ident = const_pool.tile([128, 128], f32)
        masks.make_identity(nc, ident[:])
        wsb = const_pool.tile([128, N_DT, NUM_EXPERTS], f32)
        nc.sync.dma_start(
            out=wsb[:], in_=w_ap.rearrange("p (dt e) -> p dt e", e=NUM_EXPERTS)
        )
        # bias as a per-partition column [64, 1] for the logitsT layout
        bsb = const_pool.tile([NUM_EXPERTS, 1], f32)
        nc.sync.dma_start(out=bsb[:], in_=b_ap.rearrange("o e -> e o"))

        # --- prologue wait-absorbers (keep PE insts at <=1 sync wait) ---
        # dummy transpose: absorbs the identity (Pool) dep into PE
        dmy_ps = lt_ps_pool.tile([NUM_EXPERTS, 128], tr_dt, tag="lt")
        nc.tensor.transpose(dmy_ps[:], trv(ident[:, 0:NUM_EXPERTS]), trv(ident[:]))
        # dummy matmul reading wsb: absorbs the W-load (DMA) dep into PE
        dmy_lg = lg_ps_pool.tile([NUM_EXPERTS, TOK_BLOCK], f32, tag="lg")
        nc.tensor.matmul(
            dmy_lg[:, 0:128], mmv(wsb[:, 0, :]), mmv(ident[:]), start=True, stop=True
        )

        for blk_rep in range(REPEAT * N_BLOCKS):
            blk = blk_rep % N_BLOCKS
            rep = blk_rep // N_BLOCKS
            t0 = blk * TOK_BLOCK
            o0 = rep * TOK_PER_CORE + t0
            xts = []
            for i in range(n_tt):
                xt = xnat_pool.tile([128, D_MODEL], f32, tag="xnat")
                nc.sync.dma_start(
                    out=xt[:], in_=x_ap[t0 + i * 128 : t0 + (i + 1) * 128, :]
                )
                xts.append(xt)

            lg_ps = lg_ps_pool.tile([NUM_EXPERTS, TOK_BLOCK], f32, tag="lg")
            for dt_i in range(N_DT):
                pool = xT_ps0_pool if dt_i == 0 else xT_ps_pool
                xT_ps = pool.tile([128, TOK_BLOCK], tr_dt, tag="xTp")
                for i in range(n_tt):
                    nc.tensor.transpose(
                        xT_ps[:, i * 128 : (i + 1) * 128],
                        trv(xts[i][:, dt_i * 128 : (dt_i + 1) * 128]),
                        trv(ident[:]),
                    )
                xT_sb = xT_sb_pool.tile([128, TOK_BLOCK], f32)
                if dt_i % 2 == 0:
                    nc.vector.tensor_copy(xT_sb[:], xT_ps[:].bitcast(f32))
                else:
                    nc.scalar.copy(xT_sb[:], xT_ps[:].bitcast(f32))
                nc.tensor.matmul(
                    lg_ps[:],
                    mmv(wsb[:, dt_i, :]),
                    mmv(xT_sb[:]),
                    start=(dt_i == 0),
                    stop=(dt_i == N_DT - 1),
                )

            # PSUM -> SBUF with bias folded in (per-partition scalar add)
            lg_sb = lg_sb_pool.tile([NUM_EXPERTS, TOK_BLOCK], f32)
            nc.vector.tensor_scalar_add(lg_sb[:], lg_ps[:], bsb[:])

            for i in range(n_tt):
                lt_ps = lt_ps_pool.tile([128, NUM_EXPERTS], f32, tag="lt")
                nc.tensor.transpose(
                    lt_ps[:],
                    lg_sb[:, i * 128 : (i + 1) * 128],
                    ident[:NUM_EXPERTS, :NUM_EXPERTS],
                )
                lg_t = epi_pool.tile([128, NUM_EXPERTS], f32, tag="lgt")
                nc.scalar.copy(lg_t[:], lt_ps[:])
                maxv = epi_pool.tile([128, 8], f32, tag="maxv")
                nc.vector.max(maxv[:], lg_t[:])
                maxi = epi_pool.tile([128, 8], u32, tag="maxi")
                nc.vector.max_index(maxi[:], maxv[:], lg_t[:])
                negv1 = epi_pool.tile([128, 1], f32, tag="negv1")
                nc.scalar.mul(negv1[:], maxv[:, 0:1], -1.0)
                e2 = epi_pool.tile([128, TOP_K], f32, tag="e2")
                ssum = epi_pool.tile([128, 1], f32, tag="ssum")
                nc.scalar.activation(
                    e2[:],
                    maxv[:, 0:TOP_K],
                    mybir.ActivationFunctionType.Exp,
                    bias=negv1[:],
                    scale=1.0,
                    accum_out=ssum[:],
                )
                rsum = epi_pool.tile([128, 1], f32, tag="rsum")
                nc.vector.reciprocal(rsum[:], ssum[:])
                probs = out_pool.tile([128, TOP_K], f32, tag="probs")
                nc.vector.tensor_scalar_mul(probs[:], e2[:], rsum[:])
                nc.sync.dma_start(
                    out=op_ap[o0 + i * 128 : o0 + (i + 1) * 128, :], in_=probs[:]
                )
                nc.sync.dma_start(
                    out=oi_ap[o0 + i * 128 : o0 + (i + 1) * 128, :],
                    in_=maxi[:, 0:TOP_K],
                )

    nc.compile()  # bacc legality passes: wait-splitting, reg alloc, DCE
    return nc


def _get_nc():
    key = (MM_DTYPE, TR_DTYPE, TOK_BLOCK, REPEAT)
    if key not in _BUILT:
        _BUILT[key] = _build_nc()
    return _BUILT[key]


def _run(x, W, b, trace=False):
    from concourse.bass_utils import run_bass_kernel_spmd

    x = np.ascontiguousarray(np.asarray(x, dtype=np.float32))
    W = np.asarray(W, dtype=np.float32)
    # permute W [4096, 64] -> [128, 32*64]: wp[p, dt*64+e] = W[dt*128+p, e]
    wp = np.ascontiguousarray(
        W.reshape(N_DT, 128, NUM_EXPERTS).transpose(1, 0, 2).reshape(128, -1)
    )
    b2 = np.ascontiguousarray(np.asarray(b, dtype=np.float32).reshape(1, NUM_EXPERTS))

    nc = _get_nc()
    shards = [x[c * TOK_PER_CORE : (c + 1) * TOK_PER_CORE] for c in range(N_CORES)]
    in_maps = [{"x": shards[c], "wp": wp, "b": b2} for c in range(N_CORES)]
    res = run_bass_kernel_spmd(nc, in_maps, list(range(N_CORES)), trace=trace)
    probs = np.concatenate(
        [np.asarray(r["probs"])[:TOK_PER_CORE] for r in res.results], axis=0
    )
    idx = np.concatenate(
        [np.asarray(r["idx"])[:TOK_PER_CORE].astype(np.int32) for r in res.results],
        axis=0,
    )
    return (probs, idx), res


def kernel(x, W, b):
    (probs, idx), _ = _run(x, W, b, trace=False)
    return probs, idx


def _warmup():
    """Build + compile the NEFF and run once with zeros so the first real
    kernel() call doesn't pay compile time."""
    try:
        z = np.zeros((NUM_TOKENS, D_MODEL), np.float32)
        zW = np.zeros((D_MODEL, NUM_EXPERTS), np.float32)
        zb = np.zeros((NUM_EXPERTS,), np.float32)
        _run(z, zW, zb, trace=False)
    except Exception as e:  # never block import on warmup problems
        import sys

        print(f"kernel warmup skipped: {type(e).__name__}: {e}", file=sys.stderr)


import os as _os

if _os.environ.get("KERNEL_NO_WARMUP") != "1":
    _warmup()
